# revision 16
# baseline (speedup 1.0000x reference)
"""ConvLSTM stack kernel for Trainium2 (8 NeuronCores, data-parallel over batch).

Problem: 3 independent ConvLSTM layers (each consumes the raw input x), each
iterated num_repeats times over its own (h, c) state.  B=64, H=W=32, CIN=64,
HID=128, K=3, SAME padding.

Design (per core, B_shard = 8 images):
  - Activations live in SBUF channels-major: [C, b, 34, 34] with a zero pad
    ring, so a 3x3 SAME conv is 9 shifted matmuls accumulating in PSUM
    (lhsT = W[dy,dx] as [Cin, Cout_block], rhs = strided pixel window).
  - conv(x, Wx) is invariant across repeats -> computed once per layer into a
    DRAM scratch ("xz", bias folded in) and streamed back during the steps.
    For the x-conv (Cin=64) two dy-taps are packed into one K=128 matmul by
    storing a row-shifted copy of x_pad on partitions 64..127.
  - Gate math: i,f,g,o are the 4 PSUM Cout blocks; DVE adds xz, ACT applies
    sigmoid/tanh, DVE forms c_new and h_new in place.
  - Layout conversion (DRAM pixel-major <-> SBUF channels-major) via PE
    transposes through PSUM at the start/end of each layer only.
"""

import os
import sys

sys.path.insert(0, "/opt/trn_rl_repo")

import numpy as np

import concourse.bass as bass
import concourse.mybir as mybir
from concourse import bacc
from concourse.masks import make_identity
from concourse.tile import TileContext

F32 = mybir.dt.float32
F32R = mybir.dt.float32r
AF = mybir.ActivationFunctionType

H = 32
W = 32
CIN = 64
HID = 128
NG = 4  # gate blocks of 128 channels: i, f, g, o
L = 3
PW = W + 2  # padded width
PH = H + 2  # padded height
NCORES = 8


def build_nc(BS=8, R=3, use_f32r=True):
    """Build the per-core Bass program. BS = images per core."""
    nc = bacc.Bacc()
    SD = F32R if use_f32r else F32  # dtype of matmul-feeding storage

    def _src(ap):
        # DRAM-side view matching an SD-typed SBUF destination (bit-identical)
        return ap.bitcast(F32R) if use_f32r else ap

    x_d = nc.declare_dram_parameter("x", [BS, H, W, CIN], F32, isOutput=False)
    hs_d = nc.declare_dram_parameter("hs", [L, BS, H, W, HID], F32, isOutput=False)
    cs_d = nc.declare_dram_parameter("cs", [L, BS, H, W, HID], F32, isOutput=False)
    wx_d = nc.declare_dram_parameter("Wx", [L, 3, 3, CIN, NG * HID], F32, isOutput=False)
    wh_d = nc.declare_dram_parameter("Wh", [L, 3, 3, HID, NG * HID], F32, isOutput=False)
    b_d = nc.declare_dram_parameter("b", [L, NG * HID], F32, isOutput=False)
    hl_d = nc.declare_dram_parameter("h_last", [BS, H, W, HID], F32, isOutput=True)
    ho_d = nc.declare_dram_parameter("hs_out", [L, BS, H, W, HID], F32, isOutput=True)
    co_d = nc.declare_dram_parameter("cs_out", [L, BS, H, W, HID], F32, isOutput=True)

    NBLK = 2 * BS  # pixel blocks of 512 (half an image) per step
    NT = (H * W) // 128  # 128-pixel tiles per image (8)

    # DRAM scratch: xz[l][:, blk, m, :] = (conv(x, Wx_l) + b_l) for that block
    xz_d = [nc.dram_tensor(f"xz{l}", [128, NBLK, NG, 512], F32) for l in range(L)]

    x_pix = x_d[:].rearrange("b h w c -> b (h w) c")
    hs_pix = hs_d[:].rearrange("l b h w c -> l b (h w) c")
    cs_pix = cs_d[:].rearrange("l b h w c -> l b (h w) c")
    ho_pix = ho_d[:].rearrange("l b h w c -> l b (h w) c")
    co_pix = co_d[:].rearrange("l b h w c -> l b (h w) c")
    hl_pix = hl_d[:].rearrange("b h w c -> b (h w) c")

    with TileContext(nc) as tc:
        with (
            tc.tile_pool(name="const", bufs=1) as const,
            tc.tile_pool(name="big", bufs=1) as big,
            tc.tile_pool(name="wpool", bufs=1) as wpool,
            tc.tile_pool(name="stg", bufs=2) as stg,
            tc.tile_pool(name="stgo", bufs=1) as stgo,
            tc.tile_pool(name="xzp", bufs=2) as xzp,
            tc.tile_pool(name="ew", bufs=2) as ew,
            tc.tile_pool(name="gpsum", bufs=6, space="PSUM") as gpsum,
            tc.tile_pool(name="tpsum", bufs=2, space="PSUM") as tpsum,
        ):
            identity = const.tile([128, 128], F32, name="identity")
            make_identity(nc, identity)
            if use_f32r:
                zcol = const.tile([128, 1], F32, name="zcol")
                nc.vector.memset(zcol, 0.0)
            bias_sb = const.tile([128, L, NG], F32, name="bias_sb")
            nc.sync.dma_start(
                out=bias_sb, in_=b_d[:].rearrange("l (m p) -> p l m", p=128)
            )

            def _zero_fill(t, n):
                # memset cannot produce f32r-typed output (walrus rejects);
                # a DVE broadcast copy from a zero fp32 column can.
                if use_f32r:
                    nc.vector.tensor_copy(
                        t.rearrange("p a b c -> p (a b c)"),
                        zcol.to_broadcast([128, n]),
                    )
                else:
                    nc.gpsimd.memset(t[:, :, :, :], 0.0)

            # x_pad buffer: partitions 0:64 hold x_pad, partitions 64:128 hold
            # x_pad shifted up one padded row (for dy=0/dy=1 tap pairing).
            xbuf = big.tile([128, BS, PH, PW], SD, name="xbuf")
            _zero_fill(xbuf, BS * PH * PW)

            for bb in range(BS):
                stx = stg.tile([128, NT, CIN], F32, name="stx", tag="stgin")
                nc.sync.dma_start(
                    out=stx, in_=x_pix[bb].rearrange("(t q) c -> q t c", q=128)
                )
                for t in range(NT):
                    r0 = 4 * t
                    psx = tpsum.tile([CIN, 128], F32, name="psx", tag="tps")
                    nc.tensor.transpose(psx, stx[:, t, :], identity)
                    src = psx.rearrange("c (a w) -> c a w", a=4)
                    nc.vector.tensor_copy(
                        xbuf[0:CIN, bb, 1 + r0 : 5 + r0, 1 : W + 1], src
                    )
                    nc.vector.tensor_copy(
                        xbuf[64 : 64 + CIN, bb, r0 : 4 + r0, 1 : W + 1], src
                    )

            for l in range(L):
                # ---- weights ----
                wxp = wpool.tile([128, 3, NG, 128], SD, name=f"wxp{l}", tag="wxp")
                nc.sync.dma_start(
                    out=wxp[0:CIN],
                    in_=_src(wx_d[:][l, 0].rearrange("k c (m n) -> c k m n", m=NG)),
                )
                nc.sync.dma_start(
                    out=wxp[CIN:128],
                    in_=_src(wx_d[:][l, 1].rearrange("k c (m n) -> c k m n", m=NG)),
                )
                wxs = wpool.tile([CIN, 3, NG, 128], SD, name=f"wxs{l}", tag="wxs")
                nc.sync.dma_start(
                    out=wxs, in_=_src(wx_d[:][l, 2].rearrange("k c (m n) -> c k m n", m=NG))
                )
                whl = wpool.tile([HID, 3, 3, NG, 128], SD, name=f"whl{l}", tag="whl")
                nc.sync.dma_start(
                    out=whl,
                    in_=_src(wh_d[:][l].rearrange("a k c (m n) -> c a k m n", m=NG)),
                )

                # ---- xz build: conv(x, Wx_l) + b_l -> DRAM ----
                for blk in range(NBLK):
                    bb, hh = blk // 2, blk % 2
                    y0 = 16 * hh
                    for m in range(NG):
                        ps = gpsum.tile([128, 512], F32, name="psg", tag="gps")
                        k = 0
                        for kx in range(3):  # dy=0 (top half) + dy=1 (bottom half)
                            nc.tensor.matmul(
                                ps,
                                wxp[:, kx, m, :],
                                xbuf[:, bb, y0 : y0 + 16, kx : kx + W],
                                start=(k == 0),
                                stop=False,
                            )
                            k += 1
                        for kx in range(3):  # dy=2 singles on partitions 0:64
                            nc.tensor.matmul(
                                ps,
                                wxs[:, kx, m, :],
                                xbuf[0:CIN, bb, y0 + 2 : y0 + 18, kx : kx + W],
                                start=False,
                                stop=(k == 5),
                            )
                            k += 1
                        xzt = ew.tile([128, 512], F32, name="xzt", tag="xzst")
                        nc.scalar.activation(
                            xzt, ps, AF.Identity, bias=bias_sb[:, l, m : m + 1]
                        )
                        nc.sync.dma_start(out=xz_d[l][:, blk, m], in_=xzt)

                # ---- initial states ----
                hbuf = big.tile([128, BS, PH, PW], SD, name=f"hbuf{l}", tag="hbuf")
                csb = big.tile([128, BS, H * W], F32, name=f"csb{l}", tag="csb")
                _zero_fill(hbuf, BS * PH * PW)
                for bb in range(BS):
                    sth = stg.tile([128, NT, HID], F32, name="sth", tag="stgin")
                    nc.sync.dma_start(
                        out=sth, in_=hs_pix[l, bb].rearrange("(t q) c -> q t c", q=128)
                    )
                    stc = stg.tile([128, NT, HID], F32, name="stc", tag="stgin")
                    nc.sync.dma_start(
                        out=stc, in_=cs_pix[l, bb].rearrange("(t q) c -> q t c", q=128)
                    )
                    for t in range(NT):
                        r0 = 4 * t
                        psh = tpsum.tile([HID, 128], F32, name="psh", tag="tps")
                        nc.tensor.transpose(psh, sth[:, t, :], identity)
                        nc.vector.tensor_copy(
                            hbuf[:, bb, 1 + r0 : 5 + r0, 1 : W + 1],
                            psh.rearrange("c (a w) -> c a w", a=4),
                        )
                        psc = tpsum.tile([HID, 128], F32, name="psc", tag="tps")
                        nc.tensor.transpose(psc, stc[:, t, :], identity)
                        nc.vector.tensor_copy(csb[:, bb, 128 * t : 128 * (t + 1)], psc)

                # ---- recurrent steps ----
                # Within a step the two half-image blocks read each other's
                # boundary rows of h, so h_new goes to temp tiles first and is
                # copied into hbuf only after both halves' matmuls are emitted.
                for r in range(R):
                    for bb in range(BS):
                        hts = []
                        for hh in range(2):
                            blk = 2 * bb + hh
                            y0 = 16 * hh
                            g = []
                            for m in range(NG):
                                xzl = xzp.tile(
                                    [128, 512], F32, name="xzl", tag="xzld", bufs=4
                                )
                                nc.sync.dma_start(out=xzl, in_=xz_d[l][:, blk, m])
                                ps = gpsum.tile([128, 512], F32, name="psg", tag="gps")
                                k = 0
                                for dy in range(3):
                                    for dx in range(3):
                                        nc.tensor.matmul(
                                            ps,
                                            whl[:, dy, dx, m, :],
                                            hbuf[
                                                :,
                                                bb,
                                                y0 + dy : y0 + dy + 16,
                                                dx : dx + W,
                                            ],
                                            start=(k == 0),
                                            stop=(k == 8),
                                        )
                                        k += 1
                                tm = ew.tile([128, 512], F32, name="tm", tag="tm")
                                nc.vector.tensor_add(tm, ps, xzl)
                                gm = ew.tile(
                                    [128, 512], F32, name=f"gm{m}", tag=f"gm{m}"
                                )
                                nc.scalar.activation(
                                    gm, tm, AF.Tanh if m == 2 else AF.Sigmoid
                                )
                                g.append(gm)
                            csl = csb[:, bb, 512 * hh + 0 : 512 * hh + 512]
                            u1 = ew.tile([128, 512], F32, name="u1", tag="u1")
                            nc.vector.tensor_mul(u1, g[1], csl)
                            u2 = ew.tile([128, 512], F32, name="u2", tag="u2")
                            nc.vector.tensor_mul(u2, g[0], g[2])
                            nc.vector.tensor_add(csl, u1, u2)
                            tct = ew.tile([128, 512], F32, name="tct", tag="tct")
                            nc.scalar.activation(tct, csl, AF.Tanh)
                            ht = ew.tile(
                                [128, 512], F32, name="ht", tag=f"ht{hh}", bufs=3
                            )
                            nc.vector.tensor_mul(ht, g[3], tct)
                            hts.append(ht)
                        if r < R - 1:
                            for hh in range(2):
                                y0 = 16 * hh
                                nc.vector.tensor_copy(
                                    hbuf[:, bb, 1 + y0 : 17 + y0, 1 : W + 1],
                                    hts[hh].rearrange("p (a w) -> p a w", a=16),
                                )
                        else:
                            # Last repeat: emit this image's outputs directly
                            # from the contiguous ht/c tiles (the matmul
                            # stationary operand must be a 1-free-dim AP, so
                            # strided hbuf slices can't be transposed).
                            soh = stgo.tile([128, NT, HID], F32, name="soh", tag="soh")
                            soc = stgo.tile([128, NT, HID], F32, name="soc", tag="soc")
                            for t in range(NT):
                                hh2, tt = divmod(t, 4)
                                poh = tpsum.tile([128, HID], F32, name="poh", tag="tps")
                                nc.tensor.transpose(
                                    poh,
                                    hts[hh2][:, 128 * tt : 128 * (tt + 1)],
                                    identity,
                                )
                                nc.vector.tensor_copy(soh[:, t, :], poh)
                                poc = tpsum.tile([128, HID], F32, name="poc", tag="tps")
                                nc.tensor.transpose(
                                    poc, csb[:, bb, 128 * t : 128 * (t + 1)], identity
                                )
                                nc.vector.tensor_copy(soc[:, t, :], poc)
                            nc.sync.dma_start(
                                out=ho_pix[l, bb].rearrange("(t q) c -> q t c", q=128),
                                in_=soh,
                            )
                            nc.sync.dma_start(
                                out=co_pix[l, bb].rearrange("(t q) c -> q t c", q=128),
                                in_=soc,
                            )
                            if l == L - 1:
                                nc.sync.dma_start(
                                    out=hl_pix[bb].rearrange("(t q) c -> q t c", q=128),
                                    in_=soh,
                                )

    if not nc.is_finalized():
        nc.finalize()
    return nc


LAST_RESULT = None


def _install_ntff_hook():
    """Provide antenv.axon_hooks (NTFF profiling) if the image lacks it."""
    import contextlib
    import ctypes
    import types

    try:
        from antenv.axon_hooks import get_axon_ntff_profile_hook  # noqa: F401

        return
    except ImportError:
        pass
    so_path = "/opt/axon/libaxon_pjrt.so"
    if not os.path.exists(so_path):
        return
    lib = ctypes.CDLL(so_path)
    if not hasattr(lib, "axon_start_nrt_profile"):
        return
    lib.axon_start_nrt_profile.argtypes = [
        ctypes.POINTER(ctypes.c_int64),
        ctypes.c_size_t,
    ]
    lib.axon_start_nrt_profile.restype = ctypes.c_int64
    lib.axon_stop_nrt_profile.argtypes = [ctypes.c_char_p]
    lib.axon_stop_nrt_profile.restype = ctypes.c_int64

    @contextlib.contextmanager
    def _hook(output_dir, device_ids):
        import jax

        jax.devices()
        if device_ids:
            ids = (ctypes.c_int64 * len(device_ids))(*device_ids)
            rc = lib.axon_start_nrt_profile(ids, len(device_ids))
        else:
            rc = lib.axon_start_nrt_profile(None, 0)
        if rc != 0:
            raise RuntimeError(f"axon_start_nrt_profile rc={rc}")
        try:
            yield
        finally:
            n = lib.axon_stop_nrt_profile(str(output_dir).encode())
            print(f"profile: {n} file(s) written to {output_dir}", file=sys.stderr)

    mod = types.ModuleType("antenv.axon_hooks")
    mod.get_axon_ntff_profile_hook = lambda: _hook
    mod.set_axon_ntff_profile_hook = lambda h: None
    sys.modules["antenv.axon_hooks"] = mod


def kernel(x, hs, cs, Wx, Wh, b, num_repeats):
    """Full-input entry point: shards batch over 8 cores, gathers full output."""
    global LAST_RESULT
    from concourse.bass_utils import run_bass_kernel_spmd

    x = np.ascontiguousarray(x, dtype=np.float32)
    hs = np.ascontiguousarray(hs, dtype=np.float32)
    cs = np.ascontiguousarray(cs, dtype=np.float32)
    Wx = np.ascontiguousarray(Wx, dtype=np.float32)
    Wh = np.ascontiguousarray(Wh, dtype=np.float32)
    b = np.ascontiguousarray(b, dtype=np.float32)
    R = int(num_repeats)
    B = x.shape[0]
    BS = B // NCORES

    use_f32r = os.environ.get("CONVLSTM_MM_DTYPE", "f32r") == "f32r"
    nc = build_nc(BS=BS, R=R, use_f32r=use_f32r)

    in_maps = []
    for c in range(NCORES):
        sl = slice(c * BS, (c + 1) * BS)
        in_maps.append(
            {
                "x": x[sl],
                "hs": hs[:, sl],
                "cs": cs[:, sl],
                "Wx": Wx,
                "Wh": Wh,
                "b": b,
            }
        )

    trace = bool(os.environ.get("KERNEL_TRACE"))
    if trace:
        _install_ntff_hook()
    res = run_bass_kernel_spmd(
        nc, in_maps, list(range(NCORES)), trace=trace
    )
    LAST_RESULT = res

    h_last = np.concatenate([res.results[c]["h_last"] for c in range(NCORES)], axis=0)
    hs_out = np.concatenate([res.results[c]["hs_out"] for c in range(NCORES)], axis=1)
    cs_out = np.concatenate([res.results[c]["cs_out"] for c in range(NCORES)], axis=1)
    return (h_last, hs_out, cs_out)


# revision 19
# speedup vs baseline: 1.0986x; 1.0986x over previous
"""ConvLSTM stack kernel for Trainium2 (8 NeuronCores, data-parallel over batch).

Problem: 3 independent ConvLSTM layers (each consumes the raw input x), each
iterated num_repeats times over its own (h, c) state.  B=64, H=W=32, CIN=64,
HID=128, K=3, SAME padding.

Design (per core, B_shard = 8 images):
  - Activations live in SBUF channels-major: [C, b, 34, 34] with a zero pad
    ring, so a 3x3 SAME conv is 9 shifted matmuls accumulating in PSUM
    (lhsT = W[dy,dx] as [Cin, Cout_block], rhs = strided pixel window).
  - conv(x, Wx) is invariant across repeats -> computed once per layer into a
    DRAM scratch ("xz", bias folded in) and streamed back during the steps.
    For the x-conv (Cin=64) two dy-taps are packed into one K=128 matmul by
    storing a row-shifted copy of x_pad on partitions 64..127.
  - Gate math: i,f,g,o are the 4 PSUM Cout blocks; DVE adds xz, ACT applies
    sigmoid/tanh, DVE forms c_new and h_new in place.
  - Layout conversion (DRAM pixel-major <-> SBUF channels-major) via PE
    transposes through PSUM at the start/end of each layer only.
"""

import os
import sys

sys.path.insert(0, "/opt/trn_rl_repo")

import numpy as np

import concourse.bass as bass
import concourse.mybir as mybir
from concourse import bacc
from concourse.masks import make_identity
from concourse.tile import TileContext

F32 = mybir.dt.float32
F32R = mybir.dt.float32r
AF = mybir.ActivationFunctionType

H = 32
W = 32
CIN = 64
HID = 128
NG = 4  # gate blocks of 128 channels: i, f, g, o
L = 3
PW = W + 2  # padded width
PH = H + 2  # padded height
NCORES = 8


def build_nc(BS=8, R=3, use_f32r=True):
    """Build the per-core Bass program. BS = images per core."""
    nc = bacc.Bacc()
    SD = F32R if use_f32r else F32  # dtype of matmul-feeding storage

    def _src(ap):
        # DRAM-side view matching an SD-typed SBUF destination (bit-identical)
        return ap.bitcast(F32R) if use_f32r else ap

    x_d = nc.declare_dram_parameter("x", [BS, H, W, CIN], F32, isOutput=False)
    hs_d = nc.declare_dram_parameter("hs", [L, BS, H, W, HID], F32, isOutput=False)
    cs_d = nc.declare_dram_parameter("cs", [L, BS, H, W, HID], F32, isOutput=False)
    wx_d = nc.declare_dram_parameter("Wx", [L, 3, 3, CIN, NG * HID], F32, isOutput=False)
    wh_d = nc.declare_dram_parameter("Wh", [L, 3, 3, HID, NG * HID], F32, isOutput=False)
    b_d = nc.declare_dram_parameter("b", [L, NG * HID], F32, isOutput=False)
    hl_d = nc.declare_dram_parameter("h_last", [BS, H, W, HID], F32, isOutput=True)
    ho_d = nc.declare_dram_parameter("hs_out", [L, BS, H, W, HID], F32, isOutput=True)
    co_d = nc.declare_dram_parameter("cs_out", [L, BS, H, W, HID], F32, isOutput=True)

    NBLK = 2 * BS  # pixel blocks of 512 (half an image) per step
    NT = (H * W) // 128  # 128-pixel tiles per image (8)

    # DRAM scratch: xz[l][:, blk, m, :] = (conv(x, Wx_l) + b_l) for that block
    xz_d = [nc.dram_tensor(f"xz{l}", [128, NBLK, NG, 512], F32) for l in range(L)]

    x_pix = x_d[:].rearrange("b h w c -> b (h w) c")
    hs_pix = hs_d[:].rearrange("l b h w c -> l b (h w) c")
    cs_pix = cs_d[:].rearrange("l b h w c -> l b (h w) c")
    ho_pix = ho_d[:].rearrange("l b h w c -> l b (h w) c")
    co_pix = co_d[:].rearrange("l b h w c -> l b (h w) c")
    hl_pix = hl_d[:].rearrange("b h w c -> b (h w) c")

    with TileContext(nc) as tc:
        with (
            tc.tile_pool(name="const", bufs=1) as const,
            tc.tile_pool(name="big", bufs=1) as big,
            tc.tile_pool(name="wpool", bufs=1) as wpool,
            tc.tile_pool(name="stg", bufs=2) as stg,
            tc.tile_pool(name="stgo", bufs=1) as stgo,
            tc.tile_pool(name="xzp", bufs=2) as xzp,
            tc.tile_pool(name="ew", bufs=2) as ew,
            tc.tile_pool(name="gpsum", bufs=6, space="PSUM") as gpsum,
            tc.tile_pool(name="tpsum", bufs=2, space="PSUM") as tpsum,
        ):
            identity = const.tile([128, 128], F32, name="identity")
            make_identity(nc, identity)
            if use_f32r:
                zcol = const.tile([128, 1], F32, name="zcol")
                nc.vector.memset(zcol, 0.0)
            bias_sb = const.tile([128, L, NG], F32, name="bias_sb")
            nc.sync.dma_start(
                out=bias_sb, in_=b_d[:].rearrange("l (m p) -> p l m", p=128)
            )

            def _zero_fill(t, n):
                # memset cannot produce f32r-typed output (walrus rejects);
                # a DVE broadcast copy from a zero fp32 column can.
                if use_f32r:
                    nc.vector.tensor_copy(
                        t.rearrange("p a b c -> p (a b c)"),
                        zcol.to_broadcast([128, n]),
                    )
                else:
                    nc.gpsimd.memset(t[:, :, :, :], 0.0)

            # x_pad buffer: partitions 0:64 hold x_pad, partitions 64:128 hold
            # x_pad shifted up one padded row (for dy=0/dy=1 tap pairing).
            xbuf = big.tile([128, BS, PH, PW], SD, name="xbuf")
            _zero_fill(xbuf, BS * PH * PW)

            for bb in range(BS):
                stx = stg.tile([128, NT, CIN], F32, name="stx", tag="stgin")
                nc.sync.dma_start(
                    out=stx, in_=x_pix[bb].rearrange("(t q) c -> q t c", q=128)
                )
                for t in range(NT):
                    r0 = 4 * t
                    psx = tpsum.tile([CIN, 128], F32, name="psx", tag="tps")
                    nc.tensor.transpose(psx, stx[:, t, :], identity)
                    src = psx.rearrange("c (a w) -> c a w", a=4)
                    nc.vector.tensor_copy(
                        xbuf[0:CIN, bb, 1 + r0 : 5 + r0, 1 : W + 1], src
                    )
                    nc.vector.tensor_copy(
                        xbuf[64 : 64 + CIN, bb, r0 : 4 + r0, 1 : W + 1], src
                    )

            for l in range(L):
                # ---- weights ----
                wxp = wpool.tile([128, 3, NG, 128], SD, name=f"wxp{l}", tag="wxp")
                nc.sync.dma_start(
                    out=wxp[0:CIN],
                    in_=_src(wx_d[:][l, 0].rearrange("k c (m n) -> c k m n", m=NG)),
                )
                nc.sync.dma_start(
                    out=wxp[CIN:128],
                    in_=_src(wx_d[:][l, 1].rearrange("k c (m n) -> c k m n", m=NG)),
                )
                wxs = wpool.tile([CIN, 3, NG, 128], SD, name=f"wxs{l}", tag="wxs")
                nc.sync.dma_start(
                    out=wxs, in_=_src(wx_d[:][l, 2].rearrange("k c (m n) -> c k m n", m=NG))
                )
                whl = wpool.tile([HID, 3, 3, NG, 128], SD, name=f"whl{l}", tag="whl")
                nc.sync.dma_start(
                    out=whl,
                    in_=_src(wh_d[:][l].rearrange("a k c (m n) -> c a k m n", m=NG)),
                )

                # ---- initial states ----
                hbuf = big.tile([128, BS, PH, PW], SD, name=f"hbuf{l}", tag="hbuf")
                csb = big.tile([128, BS, H * W], F32, name=f"csb{l}", tag="csb")
                _zero_fill(hbuf, BS * PH * PW)
                for bb in range(BS):
                    sth = stg.tile([128, NT, HID], F32, name="sth", tag="stgin")
                    nc.sync.dma_start(
                        out=sth, in_=hs_pix[l, bb].rearrange("(t q) c -> q t c", q=128)
                    )
                    stc = stg.tile([128, NT, HID], F32, name="stc", tag="stgin")
                    nc.sync.dma_start(
                        out=stc, in_=cs_pix[l, bb].rearrange("(t q) c -> q t c", q=128)
                    )
                    for t in range(NT):
                        r0 = 4 * t
                        psh = tpsum.tile([HID, 128], F32, name="psh", tag="tps")
                        nc.tensor.transpose(psh, sth[:, t, :], identity)
                        nc.vector.tensor_copy(
                            hbuf[:, bb, 1 + r0 : 5 + r0, 1 : W + 1],
                            psh.rearrange("c (a w) -> c a w", a=4),
                        )
                        psc = tpsum.tile([HID, 128], F32, name="psc", tag="tps")
                        nc.tensor.transpose(psc, stc[:, t, :], identity)
                        nc.vector.tensor_copy(csb[:, bb, 128 * t : 128 * (t + 1)], psc)

                # ---- recurrent steps ----
                # Within a step the two half-image blocks read each other's
                # boundary rows of h, so h_new goes to temp tiles first and is
                # copied into hbuf only after both halves' matmuls are emitted.
                for r in range(R):
                    for bb in range(BS):
                        hts = []
                        for hh in range(2):
                            blk = 2 * bb + hh
                            y0 = 16 * hh
                            g = []
                            if r == 0:
                                # Step 0 fuses the x-conv into the same PSUM
                                # accumulation group as the h-conv; the x-only
                                # partial (+bias) is snapshotted to DRAM as xz
                                # for the later repeats.
                                pss = []
                                for m in range(NG):
                                    ps = gpsum.tile(
                                        [128, 512], F32, name="psg", tag="gps"
                                    )
                                    k = 0
                                    for kx in range(3):  # dy=0/1 packed pair
                                        nc.tensor.matmul(
                                            ps,
                                            wxp[:, kx, m, :],
                                            xbuf[:, bb, y0 : y0 + 16, kx : kx + W],
                                            start=(k == 0),
                                            stop=False,
                                        )
                                        k += 1
                                    for kx in range(3):  # dy=2 singles
                                        nc.tensor.matmul(
                                            ps,
                                            wxs[:, kx, m, :],
                                            xbuf[0:CIN, bb, y0 + 2 : y0 + 18, kx : kx + W],
                                            start=False,
                                            stop=(k == 5),
                                        )
                                        k += 1
                                    pss.append(ps)
                                if R > 1:
                                    for m in range(NG):
                                        xzt = ew.tile(
                                            [128, 512], F32, name="xzt", tag="xzst"
                                        )
                                        nc.scalar.activation(
                                            xzt,
                                            pss[m],
                                            AF.Identity,
                                            bias=bias_sb[:, l, m : m + 1],
                                        )
                                        nc.sync.dma_start(
                                            out=xz_d[l][:, blk, m], in_=xzt
                                        )
                                for m in range(NG):
                                    ps = pss[m]
                                    k = 0
                                    for dy in range(3):
                                        for dx in range(3):
                                            nc.tensor.matmul(
                                                ps,
                                                whl[:, dy, dx, m, :],
                                                hbuf[
                                                    :,
                                                    bb,
                                                    y0 + dy : y0 + dy + 16,
                                                    dx : dx + W,
                                                ],
                                                start=False,
                                                stop=(k == 8),
                                                skip_group_check=True,
                                            )
                                            k += 1
                                    gm = ew.tile(
                                        [128, 512], F32, name=f"gm{m}", tag=f"gm{m}"
                                    )
                                    nc.scalar.activation(
                                        gm,
                                        ps,
                                        AF.Tanh if m == 2 else AF.Sigmoid,
                                        bias=bias_sb[:, l, m : m + 1],
                                    )
                                    g.append(gm)
                            else:
                                for m in range(NG):
                                    xzl = xzp.tile(
                                        [128, 512], F32, name="xzl", tag="xzld", bufs=4
                                    )
                                    nc.sync.dma_start(out=xzl, in_=xz_d[l][:, blk, m])
                                    ps = gpsum.tile(
                                        [128, 512], F32, name="psg", tag="gps"
                                    )
                                    k = 0
                                    for dy in range(3):
                                        for dx in range(3):
                                            nc.tensor.matmul(
                                                ps,
                                                whl[:, dy, dx, m, :],
                                                hbuf[
                                                    :,
                                                    bb,
                                                    y0 + dy : y0 + dy + 16,
                                                    dx : dx + W,
                                                ],
                                                start=(k == 0),
                                                stop=(k == 8),
                                            )
                                            k += 1
                                    tm = ew.tile([128, 512], F32, name="tm", tag="tm")
                                    nc.vector.tensor_add(tm, ps, xzl)
                                    gm = ew.tile(
                                        [128, 512], F32, name=f"gm{m}", tag=f"gm{m}"
                                    )
                                    nc.scalar.activation(
                                        gm, tm, AF.Tanh if m == 2 else AF.Sigmoid
                                    )
                                    g.append(gm)
                            csl = csb[:, bb, 512 * hh + 0 : 512 * hh + 512]
                            u1 = ew.tile([128, 512], F32, name="u1", tag="u1")
                            nc.vector.tensor_mul(u1, g[1], csl)
                            u2 = ew.tile([128, 512], F32, name="u2", tag="u2")
                            nc.vector.tensor_mul(u2, g[0], g[2])
                            nc.vector.tensor_add(csl, u1, u2)
                            tct = ew.tile([128, 512], F32, name="tct", tag="tct")
                            nc.scalar.activation(tct, csl, AF.Tanh)
                            ht = ew.tile(
                                [128, 512], F32, name="ht", tag=f"ht{hh}", bufs=3
                            )
                            nc.vector.tensor_mul(ht, g[3], tct)
                            hts.append(ht)
                        if r < R - 1:
                            for hh in range(2):
                                y0 = 16 * hh
                                nc.vector.tensor_copy(
                                    hbuf[:, bb, 1 + y0 : 17 + y0, 1 : W + 1],
                                    hts[hh].rearrange("p (a w) -> p a w", a=16),
                                )
                        else:
                            # Last repeat: emit this image's outputs directly
                            # from the contiguous ht/c tiles (the matmul
                            # stationary operand must be a 1-free-dim AP, so
                            # strided hbuf slices can't be transposed).
                            soh = stgo.tile([128, NT, HID], F32, name="soh", tag="soh")
                            soc = stgo.tile([128, NT, HID], F32, name="soc", tag="soc")
                            for t in range(NT):
                                hh2, tt = divmod(t, 4)
                                poh = tpsum.tile([128, HID], F32, name="poh", tag="tps")
                                nc.tensor.transpose(
                                    poh,
                                    hts[hh2][:, 128 * tt : 128 * (tt + 1)],
                                    identity,
                                )
                                nc.vector.tensor_copy(soh[:, t, :], poh)
                                poc = tpsum.tile([128, HID], F32, name="poc", tag="tps")
                                nc.tensor.transpose(
                                    poc, csb[:, bb, 128 * t : 128 * (t + 1)], identity
                                )
                                nc.vector.tensor_copy(soc[:, t, :], poc)
                            nc.sync.dma_start(
                                out=ho_pix[l, bb].rearrange("(t q) c -> q t c", q=128),
                                in_=soh,
                            )
                            nc.sync.dma_start(
                                out=co_pix[l, bb].rearrange("(t q) c -> q t c", q=128),
                                in_=soc,
                            )
                            if l == L - 1:
                                nc.sync.dma_start(
                                    out=hl_pix[bb].rearrange("(t q) c -> q t c", q=128),
                                    in_=soh,
                                )

    if not nc.is_finalized():
        nc.finalize()
    return nc


LAST_RESULT = None


def _install_ntff_hook():
    """Provide antenv.axon_hooks (NTFF profiling) if the image lacks it."""
    import contextlib
    import ctypes
    import types

    try:
        from antenv.axon_hooks import get_axon_ntff_profile_hook  # noqa: F401

        return
    except ImportError:
        pass
    so_path = "/opt/axon/libaxon_pjrt.so"
    if not os.path.exists(so_path):
        return
    lib = ctypes.CDLL(so_path)
    if not hasattr(lib, "axon_start_nrt_profile"):
        return
    lib.axon_start_nrt_profile.argtypes = [
        ctypes.POINTER(ctypes.c_int64),
        ctypes.c_size_t,
    ]
    lib.axon_start_nrt_profile.restype = ctypes.c_int64
    lib.axon_stop_nrt_profile.argtypes = [ctypes.c_char_p]
    lib.axon_stop_nrt_profile.restype = ctypes.c_int64

    @contextlib.contextmanager
    def _hook(output_dir, device_ids):
        import jax

        jax.devices()
        if device_ids:
            ids = (ctypes.c_int64 * len(device_ids))(*device_ids)
            rc = lib.axon_start_nrt_profile(ids, len(device_ids))
        else:
            rc = lib.axon_start_nrt_profile(None, 0)
        if rc != 0:
            raise RuntimeError(f"axon_start_nrt_profile rc={rc}")
        try:
            yield
        finally:
            n = lib.axon_stop_nrt_profile(str(output_dir).encode())
            print(f"profile: {n} file(s) written to {output_dir}", file=sys.stderr)

    mod = types.ModuleType("antenv.axon_hooks")
    mod.get_axon_ntff_profile_hook = lambda: _hook
    mod.set_axon_ntff_profile_hook = lambda h: None
    sys.modules["antenv.axon_hooks"] = mod


def kernel(x, hs, cs, Wx, Wh, b, num_repeats):
    """Full-input entry point: shards batch over 8 cores, gathers full output."""
    global LAST_RESULT
    from concourse.bass_utils import run_bass_kernel_spmd

    x = np.ascontiguousarray(x, dtype=np.float32)
    hs = np.ascontiguousarray(hs, dtype=np.float32)
    cs = np.ascontiguousarray(cs, dtype=np.float32)
    Wx = np.ascontiguousarray(Wx, dtype=np.float32)
    Wh = np.ascontiguousarray(Wh, dtype=np.float32)
    b = np.ascontiguousarray(b, dtype=np.float32)
    R = int(num_repeats)
    B = x.shape[0]
    BS = B // NCORES

    use_f32r = os.environ.get("CONVLSTM_MM_DTYPE", "f32r") == "f32r"
    nc = build_nc(BS=BS, R=R, use_f32r=use_f32r)

    in_maps = []
    for c in range(NCORES):
        sl = slice(c * BS, (c + 1) * BS)
        in_maps.append(
            {
                "x": x[sl],
                "hs": hs[:, sl],
                "cs": cs[:, sl],
                "Wx": Wx,
                "Wh": Wh,
                "b": b,
            }
        )

    trace = bool(os.environ.get("KERNEL_TRACE"))
    if trace:
        _install_ntff_hook()
    res = run_bass_kernel_spmd(
        nc, in_maps, list(range(NCORES)), trace=trace
    )
    LAST_RESULT = res

    h_last = np.concatenate([res.results[c]["h_last"] for c in range(NCORES)], axis=0)
    hs_out = np.concatenate([res.results[c]["hs_out"] for c in range(NCORES)], axis=1)
    cs_out = np.concatenate([res.results[c]["cs_out"] for c in range(NCORES)], axis=1)
    return (h_last, hs_out, cs_out)


# revision 23
# speedup vs baseline: 1.1199x; 1.0194x over previous
"""ConvLSTM stack kernel for Trainium2 (8 NeuronCores, data-parallel over batch).

Problem: 3 independent ConvLSTM layers (each consumes the raw input x), each
iterated num_repeats times over its own (h, c) state.  B=64, H=W=32, CIN=64,
HID=128, K=3, SAME padding.

Design (per core, B_shard = 8 images):
  - Activations live in SBUF channels-major: [C, b, 34, 34] with a zero pad
    ring, so a 3x3 SAME conv is 9 shifted matmuls accumulating in PSUM
    (lhsT = W[dy,dx] as [Cin, Cout_block], rhs = strided pixel window).
  - conv(x, Wx) is invariant across repeats -> computed once per layer into a
    DRAM scratch ("xz", bias folded in) and streamed back during the steps.
    For the x-conv (Cin=64) two dy-taps are packed into one K=128 matmul by
    storing a row-shifted copy of x_pad on partitions 64..127.
  - Gate math: i,f,g,o are the 4 PSUM Cout blocks; DVE adds xz, ACT applies
    sigmoid/tanh, DVE forms c_new and h_new in place.
  - Layout conversion (DRAM pixel-major <-> SBUF channels-major) via PE
    transposes through PSUM at the start/end of each layer only.
"""

import os
import sys

sys.path.insert(0, "/opt/trn_rl_repo")

import numpy as np

import concourse.bass as bass
import concourse.mybir as mybir
from concourse import bacc
from concourse.masks import make_identity
from concourse.tile import TileContext

F32 = mybir.dt.float32
F32R = mybir.dt.float32r
AF = mybir.ActivationFunctionType

H = 32
W = 32
CIN = 64
HID = 128
NG = 4  # gate blocks of 128 channels: i, f, g, o
L = 3
PW = W + 2  # padded width
PH = H + 2  # padded height
NCORES = 8


def build_nc(BS=8, R=3, use_f32r=True):
    """Build the per-core Bass program. BS = images per core."""
    nc = bacc.Bacc()
    SD = F32R if use_f32r else F32  # dtype of matmul-feeding storage

    def _src(ap):
        # DRAM-side view matching an SD-typed SBUF destination (bit-identical)
        return ap.bitcast(F32R) if use_f32r else ap

    x_d = nc.declare_dram_parameter("x", [BS, H, W, CIN], F32, isOutput=False)
    hs_d = nc.declare_dram_parameter("hs", [L, BS, H, W, HID], F32, isOutput=False)
    cs_d = nc.declare_dram_parameter("cs", [L, BS, H, W, HID], F32, isOutput=False)
    wx_d = nc.declare_dram_parameter("Wx", [L, 3, 3, CIN, NG * HID], F32, isOutput=False)
    wh_d = nc.declare_dram_parameter("Wh", [L, 3, 3, HID, NG * HID], F32, isOutput=False)
    b_d = nc.declare_dram_parameter("b", [L, NG * HID], F32, isOutput=False)
    hl_d = nc.declare_dram_parameter("h_last", [BS, H, W, HID], F32, isOutput=True)
    ho_d = nc.declare_dram_parameter("hs_out", [L, BS, H, W, HID], F32, isOutput=True)
    co_d = nc.declare_dram_parameter("cs_out", [L, BS, H, W, HID], F32, isOutput=True)

    NBLK = 2 * BS  # pixel blocks of 512 (half an image) per step
    NT = (H * W) // 128  # 128-pixel tiles per image (8)

    # DRAM scratch: xz[l][:, blk, m, :] = (conv(x, Wx_l) + b_l) for that block
    xz_d = [nc.dram_tensor(f"xz{l}", [128, NBLK, NG, 512], F32) for l in range(L)]

    x_pix = x_d[:].rearrange("b h w c -> b (h w) c")
    hs_pix = hs_d[:].rearrange("l b h w c -> l b (h w) c")
    cs_pix = cs_d[:].rearrange("l b h w c -> l b (h w) c")
    ho_pix = ho_d[:].rearrange("l b h w c -> l b (h w) c")
    co_pix = co_d[:].rearrange("l b h w c -> l b (h w) c")
    hl_pix = hl_d[:].rearrange("b h w c -> b (h w) c")

    with TileContext(nc) as tc:
        with (
            tc.tile_pool(name="const", bufs=1) as const,
            tc.tile_pool(name="big", bufs=1) as big,
            tc.tile_pool(name="wpool", bufs=1) as wpool,
            tc.tile_pool(name="stg", bufs=4) as stg,
            tc.tile_pool(name="stgo", bufs=1) as stgo,
            tc.tile_pool(name="xzp", bufs=2) as xzp,
            tc.tile_pool(name="ew", bufs=2) as ew,
            tc.tile_pool(name="gpsum", bufs=6, space="PSUM") as gpsum,
            tc.tile_pool(name="tpsum", bufs=2, space="PSUM") as tpsum,
        ):
            identity = const.tile([128, 128], F32, name="identity")
            make_identity(nc, identity)
            if use_f32r:
                zcol = const.tile([128, 1], F32, name="zcol")
                nc.vector.memset(zcol, 0.0)
            bias_sb = const.tile([128, L, NG], F32, name="bias_sb")
            nc.sync.dma_start(
                out=bias_sb, in_=b_d[:].rearrange("l (m p) -> p l m", p=128)
            )

            def _zero_fill(t, n):
                # memset cannot produce f32r-typed output (walrus rejects);
                # a DVE broadcast copy from a zero fp32 column can.
                if use_f32r:
                    nc.vector.tensor_copy(
                        t.rearrange("p a b c -> p (a b c)"),
                        zcol.to_broadcast([128, n]),
                    )
                else:
                    nc.gpsimd.memset(t[:, :, :, :], 0.0)

            # x_pad buffer: partitions 0:64 hold x_pad, partitions 64:128 hold
            # x_pad shifted up one padded row (for dy=0/dy=1 tap pairing).
            xbuf = big.tile([128, BS, PH, PW], SD, name="xbuf")
            _zero_fill(xbuf, BS * PH * PW)

            def _fill_x(bb):
                stx = stg.tile([128, NT, CIN], F32, name="stx", tag="stgin")
                nc.sync.dma_start(
                    out=stx, in_=x_pix[bb].rearrange("(t q) c -> q t c", q=128)
                )
                for t in range(NT):
                    r0 = 4 * t
                    psx = tpsum.tile([CIN, 128], F32, name="psx", tag="tps")
                    nc.tensor.transpose(psx, stx[:, t, :], identity)
                    src = psx.rearrange("c (a w) -> c a w", a=4)
                    nc.vector.tensor_copy(
                        xbuf[0:CIN, bb, 1 + r0 : 5 + r0, 1 : W + 1], src
                    )
                    nc.vector.tensor_copy(
                        xbuf[64 : 64 + CIN, bb, r0 : 4 + r0, 1 : W + 1], src
                    )

            for l in range(L):
                # ---- weights ----
                wxp = wpool.tile([128, 3, NG, 128], SD, name=f"wxp{l}", tag="wxp")
                nc.sync.dma_start(
                    out=wxp[0:CIN],
                    in_=_src(wx_d[:][l, 0].rearrange("k c (m n) -> c k m n", m=NG)),
                )
                nc.sync.dma_start(
                    out=wxp[CIN:128],
                    in_=_src(wx_d[:][l, 1].rearrange("k c (m n) -> c k m n", m=NG)),
                )
                wxs = wpool.tile([CIN, 3, NG, 128], SD, name=f"wxs{l}", tag="wxs")
                nc.sync.dma_start(
                    out=wxs, in_=_src(wx_d[:][l, 2].rearrange("k c (m n) -> c k m n", m=NG))
                )
                whl = wpool.tile([HID, 3, 3, NG, 128], SD, name=f"whl{l}", tag="whl")
                nc.sync.dma_start(
                    out=whl,
                    in_=_src(wh_d[:][l].rearrange("a k c (m n) -> c a k m n", m=NG)),
                )

                # ---- initial states ----
                hbuf = big.tile([128, BS, PH, PW], SD, name=f"hbuf{l}", tag="hbuf")
                csb = big.tile([128, BS, H * W], F32, name=f"csb{l}", tag="csb")
                _zero_fill(hbuf, BS * PH * PW)
                for bb in range(BS):
                    if l == 0:
                        _fill_x(bb)
                    sth = stg.tile([128, NT, HID], F32, name="sth", tag="stgin")
                    nc.sync.dma_start(
                        out=sth, in_=hs_pix[l, bb].rearrange("(t q) c -> q t c", q=128)
                    )
                    stc = stg.tile([128, NT, HID], F32, name="stc", tag="stgin")
                    nc.sync.dma_start(
                        out=stc, in_=cs_pix[l, bb].rearrange("(t q) c -> q t c", q=128)
                    )
                    for t in range(NT):
                        r0 = 4 * t
                        psh = tpsum.tile([HID, 128], F32, name="psh", tag="tps")
                        nc.tensor.transpose(psh, sth[:, t, :], identity)
                        nc.vector.tensor_copy(
                            hbuf[:, bb, 1 + r0 : 5 + r0, 1 : W + 1],
                            psh.rearrange("c (a w) -> c a w", a=4),
                        )
                        psc = tpsum.tile([HID, 128], F32, name="psc", tag="tps")
                        nc.tensor.transpose(psc, stc[:, t, :], identity)
                        nc.vector.tensor_copy(csb[:, bb, 128 * t : 128 * (t + 1)], psc)

                # ---- recurrent steps ----
                # Within a step the two half-image blocks read each other's
                # boundary rows of h, so h_new goes to temp tiles first and is
                # copied into hbuf only after both halves' matmuls are emitted.
                for r in range(R):
                    for bb in range(BS):
                        hts = []
                        for hh in range(2):
                            blk = 2 * bb + hh
                            y0 = 16 * hh
                            g = []
                            if r == 0:
                                # Step 0 fuses the x-conv into the same PSUM
                                # accumulation group as the h-conv; the x-only
                                # partial (+bias) is snapshotted to DRAM as xz
                                # for the later repeats.
                                pss = []
                                for m in range(NG):
                                    ps = gpsum.tile(
                                        [128, 512], F32, name="psg", tag="gps"
                                    )
                                    k = 0
                                    for kx in range(3):  # dy=0/1 packed pair
                                        nc.tensor.matmul(
                                            ps,
                                            wxp[:, kx, m, :],
                                            xbuf[:, bb, y0 : y0 + 16, kx : kx + W],
                                            start=(k == 0),
                                            stop=False,
                                        )
                                        k += 1
                                    for kx in range(3):  # dy=2 singles
                                        nc.tensor.matmul(
                                            ps,
                                            wxs[:, kx, m, :],
                                            xbuf[0:CIN, bb, y0 + 2 : y0 + 18, kx : kx + W],
                                            start=False,
                                            stop=(k == 5),
                                        )
                                        k += 1
                                    pss.append(ps)
                                if R > 1:
                                    for m in range(NG):
                                        xzt = ew.tile(
                                            [128, 512], F32, name="xzt", tag="xzst"
                                        )
                                        nc.scalar.activation(
                                            xzt,
                                            pss[m],
                                            AF.Identity,
                                            bias=bias_sb[:, l, m : m + 1],
                                        )
                                        nc.sync.dma_start(
                                            out=xz_d[l][:, blk, m], in_=xzt
                                        )
                                for m in range(NG):
                                    ps = pss[m]
                                    k = 0
                                    for dy in range(3):
                                        for dx in range(3):
                                            nc.tensor.matmul(
                                                ps,
                                                whl[:, dy, dx, m, :],
                                                hbuf[
                                                    :,
                                                    bb,
                                                    y0 + dy : y0 + dy + 16,
                                                    dx : dx + W,
                                                ],
                                                start=False,
                                                stop=(k == 8),
                                                skip_group_check=True,
                                            )
                                            k += 1
                                    gm = ew.tile(
                                        [128, 512], F32, name=f"gm{m}", tag=f"gm{m}"
                                    )
                                    nc.scalar.activation(
                                        gm,
                                        ps,
                                        AF.Tanh if m == 2 else AF.Sigmoid,
                                        bias=bias_sb[:, l, m : m + 1],
                                    )
                                    g.append(gm)
                            else:
                                for m in range(NG):
                                    xzl = xzp.tile(
                                        [128, 512], F32, name="xzl", tag="xzld", bufs=4
                                    )
                                    nc.sync.dma_start(out=xzl, in_=xz_d[l][:, blk, m])
                                    ps = gpsum.tile(
                                        [128, 512], F32, name="psg", tag="gps"
                                    )
                                    k = 0
                                    for dy in range(3):
                                        for dx in range(3):
                                            nc.tensor.matmul(
                                                ps,
                                                whl[:, dy, dx, m, :],
                                                hbuf[
                                                    :,
                                                    bb,
                                                    y0 + dy : y0 + dy + 16,
                                                    dx : dx + W,
                                                ],
                                                start=(k == 0),
                                                stop=(k == 8),
                                            )
                                            k += 1
                                    tm = ew.tile([128, 512], F32, name="tm", tag="tm")
                                    nc.vector.tensor_add(tm, ps, xzl)
                                    gm = ew.tile(
                                        [128, 512], F32, name=f"gm{m}", tag=f"gm{m}"
                                    )
                                    nc.scalar.activation(
                                        gm, tm, AF.Tanh if m == 2 else AF.Sigmoid
                                    )
                                    g.append(gm)
                            csl = csb[:, bb, 512 * hh + 0 : 512 * hh + 512]
                            u1 = ew.tile([128, 512], F32, name="u1", tag="u1", bufs=1)
                            nc.vector.tensor_mul(u1, g[1], csl)
                            u2 = ew.tile([128, 512], F32, name="u2", tag="u2", bufs=1)
                            nc.vector.tensor_mul(u2, g[0], g[2])
                            nc.vector.tensor_add(csl, u1, u2)
                            tct = ew.tile([128, 512], F32, name="tct", tag="tct", bufs=1)
                            nc.scalar.activation(tct, csl, AF.Tanh)
                            ht = ew.tile(
                                [128, 512], F32, name="ht", tag=f"ht{hh}", bufs=2
                            )
                            nc.vector.tensor_mul(ht, g[3], tct)
                            hts.append(ht)
                        if r < R - 1:
                            for hh in range(2):
                                y0 = 16 * hh
                                nc.vector.tensor_copy(
                                    hbuf[:, bb, 1 + y0 : 17 + y0, 1 : W + 1],
                                    hts[hh].rearrange("p (a w) -> p a w", a=16),
                                )
                        else:
                            # Last repeat: emit this image's outputs directly
                            # from the contiguous ht/c tiles (the matmul
                            # stationary operand must be a 1-free-dim AP, so
                            # strided hbuf slices can't be transposed).
                            soh = stgo.tile([128, NT, HID], F32, name="soh", tag="soh")
                            soc = stgo.tile([128, NT, HID], F32, name="soc", tag="soc")
                            for t in range(NT):
                                hh2, tt = divmod(t, 4)
                                poh = tpsum.tile([128, HID], F32, name="poh", tag="tps")
                                nc.tensor.transpose(
                                    poh,
                                    hts[hh2][:, 128 * tt : 128 * (tt + 1)],
                                    identity,
                                )
                                nc.vector.tensor_copy(soh[:, t, :], poh)
                                poc = tpsum.tile([128, HID], F32, name="poc", tag="tps")
                                nc.tensor.transpose(
                                    poc, csb[:, bb, 128 * t : 128 * (t + 1)], identity
                                )
                                nc.vector.tensor_copy(soc[:, t, :], poc)
                            nc.sync.dma_start(
                                out=ho_pix[l, bb].rearrange("(t q) c -> q t c", q=128),
                                in_=soh,
                            )
                            nc.sync.dma_start(
                                out=co_pix[l, bb].rearrange("(t q) c -> q t c", q=128),
                                in_=soc,
                            )
                            if l == L - 1:
                                nc.sync.dma_start(
                                    out=hl_pix[bb].rearrange("(t q) c -> q t c", q=128),
                                    in_=soh,
                                )

    if not nc.is_finalized():
        nc.finalize()
    return nc


LAST_RESULT = None


def _install_ntff_hook():
    """Provide antenv.axon_hooks (NTFF profiling) if the image lacks it."""
    import contextlib
    import ctypes
    import types

    try:
        from antenv.axon_hooks import get_axon_ntff_profile_hook  # noqa: F401

        return
    except ImportError:
        pass
    so_path = "/opt/axon/libaxon_pjrt.so"
    if not os.path.exists(so_path):
        return
    lib = ctypes.CDLL(so_path)
    if not hasattr(lib, "axon_start_nrt_profile"):
        return
    lib.axon_start_nrt_profile.argtypes = [
        ctypes.POINTER(ctypes.c_int64),
        ctypes.c_size_t,
    ]
    lib.axon_start_nrt_profile.restype = ctypes.c_int64
    lib.axon_stop_nrt_profile.argtypes = [ctypes.c_char_p]
    lib.axon_stop_nrt_profile.restype = ctypes.c_int64

    @contextlib.contextmanager
    def _hook(output_dir, device_ids):
        import jax

        jax.devices()
        if device_ids:
            ids = (ctypes.c_int64 * len(device_ids))(*device_ids)
            rc = lib.axon_start_nrt_profile(ids, len(device_ids))
        else:
            rc = lib.axon_start_nrt_profile(None, 0)
        if rc != 0:
            raise RuntimeError(f"axon_start_nrt_profile rc={rc}")
        try:
            yield
        finally:
            n = lib.axon_stop_nrt_profile(str(output_dir).encode())
            print(f"profile: {n} file(s) written to {output_dir}", file=sys.stderr)

    mod = types.ModuleType("antenv.axon_hooks")
    mod.get_axon_ntff_profile_hook = lambda: _hook
    mod.set_axon_ntff_profile_hook = lambda h: None
    sys.modules["antenv.axon_hooks"] = mod


def kernel(x, hs, cs, Wx, Wh, b, num_repeats):
    """Full-input entry point: shards batch over 8 cores, gathers full output."""
    global LAST_RESULT
    from concourse.bass_utils import run_bass_kernel_spmd

    x = np.ascontiguousarray(x, dtype=np.float32)
    hs = np.ascontiguousarray(hs, dtype=np.float32)
    cs = np.ascontiguousarray(cs, dtype=np.float32)
    Wx = np.ascontiguousarray(Wx, dtype=np.float32)
    Wh = np.ascontiguousarray(Wh, dtype=np.float32)
    b = np.ascontiguousarray(b, dtype=np.float32)
    R = int(num_repeats)
    B = x.shape[0]
    BS = B // NCORES

    use_f32r = os.environ.get("CONVLSTM_MM_DTYPE", "f32r") == "f32r"
    nc = build_nc(BS=BS, R=R, use_f32r=use_f32r)

    in_maps = []
    for c in range(NCORES):
        sl = slice(c * BS, (c + 1) * BS)
        in_maps.append(
            {
                "x": x[sl],
                "hs": hs[:, sl],
                "cs": cs[:, sl],
                "Wx": Wx,
                "Wh": Wh,
                "b": b,
            }
        )

    trace = bool(os.environ.get("KERNEL_TRACE"))
    if trace:
        _install_ntff_hook()
    res = run_bass_kernel_spmd(
        nc, in_maps, list(range(NCORES)), trace=trace
    )
    LAST_RESULT = res

    h_last = np.concatenate([res.results[c]["h_last"] for c in range(NCORES)], axis=0)
    hs_out = np.concatenate([res.results[c]["hs_out"] for c in range(NCORES)], axis=1)
    cs_out = np.concatenate([res.results[c]["cs_out"] for c in range(NCORES)], axis=1)
    return (h_last, hs_out, cs_out)


# revision 24
# speedup vs baseline: 1.2467x; 1.1132x over previous
"""ConvLSTM stack kernel for Trainium2 (8 NeuronCores, data-parallel over batch).

Problem: 3 independent ConvLSTM layers (each consumes the raw input x), each
iterated num_repeats times over its own (h, c) state.  B=64, H=W=32, CIN=64,
HID=128, K=3, SAME padding.

Design (per core, B_shard = 8 images):
  - Activations live in SBUF channels-major: [C, b, 34, 34] with a zero pad
    ring, so a 3x3 SAME conv is 9 shifted matmuls accumulating in PSUM
    (lhsT = W[dy,dx] as [Cin, Cout_block], rhs = strided pixel window).
  - conv(x, Wx) is invariant across repeats -> computed once per layer into a
    DRAM scratch ("xz", bias folded in) and streamed back during the steps.
    For the x-conv (Cin=64) two dy-taps are packed into one K=128 matmul by
    storing a row-shifted copy of x_pad on partitions 64..127.
  - Gate math: i,f,g,o are the 4 PSUM Cout blocks; DVE adds xz, ACT applies
    sigmoid/tanh, DVE forms c_new and h_new in place.
  - Layout conversion (DRAM pixel-major <-> SBUF channels-major) via PE
    transposes through PSUM at the start/end of each layer only.
"""

import os
import sys

sys.path.insert(0, "/opt/trn_rl_repo")

import numpy as np

import concourse.bass as bass
import concourse.mybir as mybir
from concourse import bacc
from concourse.masks import make_identity
from concourse.tile import TileContext

F32 = mybir.dt.float32
F32R = mybir.dt.float32r
BF16 = mybir.dt.bfloat16
AF = mybir.ActivationFunctionType

H = 32
W = 32
CIN = 64
HID = 128
NG = 4  # gate blocks of 128 channels: i, f, g, o
L = 3
PW = W + 2  # padded width
PH = H + 2  # padded height
NCORES = 8


def build_nc(BS=8, R=3, mm_dtype="f32r"):
    """Build the per-core Bass program. BS = images per core."""
    nc = bacc.Bacc()
    use_f32r = mm_dtype == "f32r"
    # dtype of matmul-feeding storage (weights, xbuf, hbuf)
    SD = {"f32r": F32R, "f32": F32, "bf16": BF16}[mm_dtype]

    def _src(ap):
        # DRAM-side view matching an SD-typed SBUF destination (bit-identical)
        return ap.bitcast(F32R) if use_f32r else ap

    def _wdma(out, in_):
        # bf16 weights need the cast-capable SWDGE path
        if mm_dtype == "bf16":
            nc.gpsimd.dma_start(out=out, in_=in_)
        else:
            nc.sync.dma_start(out=out, in_=_src(in_))

    x_d = nc.declare_dram_parameter("x", [BS, H, W, CIN], F32, isOutput=False)
    hs_d = nc.declare_dram_parameter("hs", [L, BS, H, W, HID], F32, isOutput=False)
    cs_d = nc.declare_dram_parameter("cs", [L, BS, H, W, HID], F32, isOutput=False)
    wx_d = nc.declare_dram_parameter("Wx", [L, 3, 3, CIN, NG * HID], F32, isOutput=False)
    wh_d = nc.declare_dram_parameter("Wh", [L, 3, 3, HID, NG * HID], F32, isOutput=False)
    b_d = nc.declare_dram_parameter("b", [L, NG * HID], F32, isOutput=False)
    hl_d = nc.declare_dram_parameter("h_last", [BS, H, W, HID], F32, isOutput=True)
    ho_d = nc.declare_dram_parameter("hs_out", [L, BS, H, W, HID], F32, isOutput=True)
    co_d = nc.declare_dram_parameter("cs_out", [L, BS, H, W, HID], F32, isOutput=True)

    NBLK = 2 * BS  # pixel blocks of 512 (half an image) per step
    NT = (H * W) // 128  # 128-pixel tiles per image (8)

    # DRAM scratch: xz[l][:, blk, m, :] = (conv(x, Wx_l) + b_l) for that block
    xz_d = [nc.dram_tensor(f"xz{l}", [128, NBLK, NG, 512], F32) for l in range(L)]

    x_pix = x_d[:].rearrange("b h w c -> b (h w) c")
    hs_pix = hs_d[:].rearrange("l b h w c -> l b (h w) c")
    cs_pix = cs_d[:].rearrange("l b h w c -> l b (h w) c")
    ho_pix = ho_d[:].rearrange("l b h w c -> l b (h w) c")
    co_pix = co_d[:].rearrange("l b h w c -> l b (h w) c")
    hl_pix = hl_d[:].rearrange("b h w c -> b (h w) c")

    with TileContext(nc) as tc:
        with (
            tc.tile_pool(name="const", bufs=1) as const,
            tc.tile_pool(name="big", bufs=1) as big,
            tc.tile_pool(name="wpool", bufs=1) as wpool,
            tc.tile_pool(name="stg", bufs=4) as stg,
            tc.tile_pool(name="stgo", bufs=1) as stgo,
            tc.tile_pool(name="xzp", bufs=2) as xzp,
            tc.tile_pool(name="ew", bufs=2) as ew,
            tc.tile_pool(name="gpsum", bufs=6, space="PSUM") as gpsum,
            tc.tile_pool(name="tpsum", bufs=2, space="PSUM") as tpsum,
        ):
            identity = const.tile([128, 128], F32, name="identity")
            make_identity(nc, identity)
            if use_f32r:
                zcol = const.tile([128, 1], F32, name="zcol")
                nc.vector.memset(zcol, 0.0)
            bias_sb = const.tile([128, L, NG], F32, name="bias_sb")
            nc.sync.dma_start(
                out=bias_sb, in_=b_d[:].rearrange("l (m p) -> p l m", p=128)
            )

            def _zero_fill(t, n):
                # memset cannot produce f32r-typed output (walrus rejects);
                # a DVE broadcast copy from a zero fp32 column can.
                if use_f32r:
                    nc.vector.tensor_copy(
                        t.rearrange("p a b c -> p (a b c)"),
                        zcol.to_broadcast([128, n]),
                    )
                else:
                    nc.gpsimd.memset(t[:, :, :, :], 0.0)

            # x_pad buffer: partitions 0:64 hold x_pad, partitions 64:128 hold
            # x_pad shifted up one padded row (for dy=0/dy=1 tap pairing).
            xbuf = big.tile([128, BS, PH, PW], SD, name="xbuf")
            _zero_fill(xbuf, BS * PH * PW)

            def _fill_x(bb):
                stx = stg.tile([128, NT, CIN], F32, name="stx", tag="stgin")
                nc.sync.dma_start(
                    out=stx, in_=x_pix[bb].rearrange("(t q) c -> q t c", q=128)
                )
                for t in range(NT):
                    r0 = 4 * t
                    psx = tpsum.tile([CIN, 128], F32, name="psx", tag="tps")
                    nc.tensor.transpose(psx, stx[:, t, :], identity)
                    src = psx.rearrange("c (a w) -> c a w", a=4)
                    nc.vector.tensor_copy(
                        xbuf[0:CIN, bb, 1 + r0 : 5 + r0, 1 : W + 1], src
                    )
                    nc.vector.tensor_copy(
                        xbuf[64 : 64 + CIN, bb, r0 : 4 + r0, 1 : W + 1], src
                    )

            for l in range(L):
                # ---- weights ----
                wxp = wpool.tile([128, 3, NG, 128], SD, name=f"wxp{l}", tag="wxp")
                _wdma(wxp[0:CIN], wx_d[:][l, 0].rearrange("k c (m n) -> c k m n", m=NG))
                _wdma(wxp[CIN:128], wx_d[:][l, 1].rearrange("k c (m n) -> c k m n", m=NG))
                wxs = wpool.tile([CIN, 3, NG, 128], SD, name=f"wxs{l}", tag="wxs")
                _wdma(wxs, wx_d[:][l, 2].rearrange("k c (m n) -> c k m n", m=NG))
                whl = wpool.tile([HID, 3, 3, NG, 128], SD, name=f"whl{l}", tag="whl")
                _wdma(whl, wh_d[:][l].rearrange("a k c (m n) -> c a k m n", m=NG))

                # ---- initial states ----
                hbuf = big.tile([128, BS, PH, PW], SD, name=f"hbuf{l}", tag="hbuf")
                csb = big.tile([128, BS, H * W], F32, name=f"csb{l}", tag="csb")
                _zero_fill(hbuf, BS * PH * PW)
                for bb in range(BS):
                    if l == 0:
                        _fill_x(bb)
                    sth = stg.tile([128, NT, HID], F32, name="sth", tag="stgin")
                    nc.sync.dma_start(
                        out=sth, in_=hs_pix[l, bb].rearrange("(t q) c -> q t c", q=128)
                    )
                    stc = stg.tile([128, NT, HID], F32, name="stc", tag="stgin")
                    nc.sync.dma_start(
                        out=stc, in_=cs_pix[l, bb].rearrange("(t q) c -> q t c", q=128)
                    )
                    for t in range(NT):
                        r0 = 4 * t
                        psh = tpsum.tile([HID, 128], F32, name="psh", tag="tps")
                        nc.tensor.transpose(psh, sth[:, t, :], identity)
                        nc.vector.tensor_copy(
                            hbuf[:, bb, 1 + r0 : 5 + r0, 1 : W + 1],
                            psh.rearrange("c (a w) -> c a w", a=4),
                        )
                        psc = tpsum.tile([HID, 128], F32, name="psc", tag="tps")
                        nc.tensor.transpose(psc, stc[:, t, :], identity)
                        nc.vector.tensor_copy(csb[:, bb, 128 * t : 128 * (t + 1)], psc)

                # ---- recurrent steps ----
                # Within a step the two half-image blocks read each other's
                # boundary rows of h, so h_new goes to temp tiles first and is
                # copied into hbuf only after both halves' matmuls are emitted.
                for r in range(R):
                    for bb in range(BS):
                        hts = []
                        for hh in range(2):
                            blk = 2 * bb + hh
                            y0 = 16 * hh
                            g = []
                            if r == 0:
                                # Step 0 fuses the x-conv into the same PSUM
                                # accumulation group as the h-conv; the x-only
                                # partial (+bias) is snapshotted to DRAM as xz
                                # for the later repeats.
                                pss = []
                                for m in range(NG):
                                    ps = gpsum.tile(
                                        [128, 512], F32, name="psg", tag="gps"
                                    )
                                    k = 0
                                    for kx in range(3):  # dy=0/1 packed pair
                                        nc.tensor.matmul(
                                            ps,
                                            wxp[:, kx, m, :],
                                            xbuf[:, bb, y0 : y0 + 16, kx : kx + W],
                                            start=(k == 0),
                                            stop=False,
                                        )
                                        k += 1
                                    for kx in range(3):  # dy=2 singles
                                        nc.tensor.matmul(
                                            ps,
                                            wxs[:, kx, m, :],
                                            xbuf[0:CIN, bb, y0 + 2 : y0 + 18, kx : kx + W],
                                            start=False,
                                            stop=(k == 5),
                                        )
                                        k += 1
                                    pss.append(ps)
                                if R > 1:
                                    for m in range(NG):
                                        xzt = ew.tile(
                                            [128, 512], F32, name="xzt", tag="xzst"
                                        )
                                        nc.scalar.activation(
                                            xzt,
                                            pss[m],
                                            AF.Identity,
                                            bias=bias_sb[:, l, m : m + 1],
                                        )
                                        nc.sync.dma_start(
                                            out=xz_d[l][:, blk, m], in_=xzt
                                        )
                                for m in range(NG):
                                    ps = pss[m]
                                    k = 0
                                    for dy in range(3):
                                        for dx in range(3):
                                            nc.tensor.matmul(
                                                ps,
                                                whl[:, dy, dx, m, :],
                                                hbuf[
                                                    :,
                                                    bb,
                                                    y0 + dy : y0 + dy + 16,
                                                    dx : dx + W,
                                                ],
                                                start=False,
                                                stop=(k == 8),
                                                skip_group_check=True,
                                            )
                                            k += 1
                                    gm = ew.tile(
                                        [128, 512], F32, name=f"gm{m}", tag=f"gm{m}"
                                    )
                                    nc.scalar.activation(
                                        gm,
                                        ps,
                                        AF.Tanh if m == 2 else AF.Sigmoid,
                                        bias=bias_sb[:, l, m : m + 1],
                                    )
                                    g.append(gm)
                            else:
                                for m in range(NG):
                                    xzl = xzp.tile(
                                        [128, 512], F32, name="xzl", tag="xzld", bufs=4
                                    )
                                    nc.sync.dma_start(out=xzl, in_=xz_d[l][:, blk, m])
                                    ps = gpsum.tile(
                                        [128, 512], F32, name="psg", tag="gps"
                                    )
                                    k = 0
                                    for dy in range(3):
                                        for dx in range(3):
                                            nc.tensor.matmul(
                                                ps,
                                                whl[:, dy, dx, m, :],
                                                hbuf[
                                                    :,
                                                    bb,
                                                    y0 + dy : y0 + dy + 16,
                                                    dx : dx + W,
                                                ],
                                                start=(k == 0),
                                                stop=(k == 8),
                                            )
                                            k += 1
                                    tm = ew.tile([128, 512], F32, name="tm", tag="tm")
                                    nc.vector.tensor_add(tm, ps, xzl)
                                    gm = ew.tile(
                                        [128, 512], F32, name=f"gm{m}", tag=f"gm{m}"
                                    )
                                    nc.scalar.activation(
                                        gm, tm, AF.Tanh if m == 2 else AF.Sigmoid
                                    )
                                    g.append(gm)
                            csl = csb[:, bb, 512 * hh + 0 : 512 * hh + 512]
                            u1 = ew.tile([128, 512], F32, name="u1", tag="u1", bufs=1)
                            nc.vector.tensor_mul(u1, g[1], csl)
                            u2 = ew.tile([128, 512], F32, name="u2", tag="u2", bufs=1)
                            nc.vector.tensor_mul(u2, g[0], g[2])
                            nc.vector.tensor_add(csl, u1, u2)
                            tct = ew.tile([128, 512], F32, name="tct", tag="tct", bufs=1)
                            nc.scalar.activation(tct, csl, AF.Tanh)
                            ht = ew.tile(
                                [128, 512], F32, name="ht", tag=f"ht{hh}", bufs=2
                            )
                            nc.vector.tensor_mul(ht, g[3], tct)
                            hts.append(ht)
                        if r < R - 1:
                            for hh in range(2):
                                y0 = 16 * hh
                                nc.vector.tensor_copy(
                                    hbuf[:, bb, 1 + y0 : 17 + y0, 1 : W + 1],
                                    hts[hh].rearrange("p (a w) -> p a w", a=16),
                                )
                        else:
                            # Last repeat: emit this image's outputs directly
                            # from the contiguous ht/c tiles (the matmul
                            # stationary operand must be a 1-free-dim AP, so
                            # strided hbuf slices can't be transposed).
                            soh = stgo.tile([128, NT, HID], F32, name="soh", tag="soh")
                            soc = stgo.tile([128, NT, HID], F32, name="soc", tag="soc")
                            for t in range(NT):
                                hh2, tt = divmod(t, 4)
                                poh = tpsum.tile([128, HID], F32, name="poh", tag="tps")
                                nc.tensor.transpose(
                                    poh,
                                    hts[hh2][:, 128 * tt : 128 * (tt + 1)],
                                    identity,
                                )
                                nc.vector.tensor_copy(soh[:, t, :], poh)
                                poc = tpsum.tile([128, HID], F32, name="poc", tag="tps")
                                nc.tensor.transpose(
                                    poc, csb[:, bb, 128 * t : 128 * (t + 1)], identity
                                )
                                nc.vector.tensor_copy(soc[:, t, :], poc)
                            nc.sync.dma_start(
                                out=ho_pix[l, bb].rearrange("(t q) c -> q t c", q=128),
                                in_=soh,
                            )
                            nc.sync.dma_start(
                                out=co_pix[l, bb].rearrange("(t q) c -> q t c", q=128),
                                in_=soc,
                            )
                            if l == L - 1:
                                nc.sync.dma_start(
                                    out=hl_pix[bb].rearrange("(t q) c -> q t c", q=128),
                                    in_=soh,
                                )

    if not nc.is_finalized():
        nc.finalize()
    return nc


LAST_RESULT = None


def _install_ntff_hook():
    """Provide antenv.axon_hooks (NTFF profiling) if the image lacks it."""
    import contextlib
    import ctypes
    import types

    try:
        from antenv.axon_hooks import get_axon_ntff_profile_hook  # noqa: F401

        return
    except ImportError:
        pass
    so_path = "/opt/axon/libaxon_pjrt.so"
    if not os.path.exists(so_path):
        return
    lib = ctypes.CDLL(so_path)
    if not hasattr(lib, "axon_start_nrt_profile"):
        return
    lib.axon_start_nrt_profile.argtypes = [
        ctypes.POINTER(ctypes.c_int64),
        ctypes.c_size_t,
    ]
    lib.axon_start_nrt_profile.restype = ctypes.c_int64
    lib.axon_stop_nrt_profile.argtypes = [ctypes.c_char_p]
    lib.axon_stop_nrt_profile.restype = ctypes.c_int64

    @contextlib.contextmanager
    def _hook(output_dir, device_ids):
        import jax

        jax.devices()
        if device_ids:
            ids = (ctypes.c_int64 * len(device_ids))(*device_ids)
            rc = lib.axon_start_nrt_profile(ids, len(device_ids))
        else:
            rc = lib.axon_start_nrt_profile(None, 0)
        if rc != 0:
            raise RuntimeError(f"axon_start_nrt_profile rc={rc}")
        try:
            yield
        finally:
            n = lib.axon_stop_nrt_profile(str(output_dir).encode())
            print(f"profile: {n} file(s) written to {output_dir}", file=sys.stderr)

    mod = types.ModuleType("antenv.axon_hooks")
    mod.get_axon_ntff_profile_hook = lambda: _hook
    mod.set_axon_ntff_profile_hook = lambda h: None
    sys.modules["antenv.axon_hooks"] = mod


def kernel(x, hs, cs, Wx, Wh, b, num_repeats):
    """Full-input entry point: shards batch over 8 cores, gathers full output."""
    global LAST_RESULT
    from concourse.bass_utils import run_bass_kernel_spmd

    x = np.ascontiguousarray(x, dtype=np.float32)
    hs = np.ascontiguousarray(hs, dtype=np.float32)
    cs = np.ascontiguousarray(cs, dtype=np.float32)
    Wx = np.ascontiguousarray(Wx, dtype=np.float32)
    Wh = np.ascontiguousarray(Wh, dtype=np.float32)
    b = np.ascontiguousarray(b, dtype=np.float32)
    R = int(num_repeats)
    B = x.shape[0]
    BS = B // NCORES

    mm_dtype = os.environ.get("CONVLSTM_MM_DTYPE", "f32r")
    nc = build_nc(BS=BS, R=R, mm_dtype=mm_dtype)

    in_maps = []
    for c in range(NCORES):
        sl = slice(c * BS, (c + 1) * BS)
        in_maps.append(
            {
                "x": x[sl],
                "hs": hs[:, sl],
                "cs": cs[:, sl],
                "Wx": Wx,
                "Wh": Wh,
                "b": b,
            }
        )

    trace = bool(os.environ.get("KERNEL_TRACE"))
    if trace:
        _install_ntff_hook()
    res = run_bass_kernel_spmd(
        nc, in_maps, list(range(NCORES)), trace=trace
    )
    LAST_RESULT = res

    h_last = np.concatenate([res.results[c]["h_last"] for c in range(NCORES)], axis=0)
    hs_out = np.concatenate([res.results[c]["hs_out"] for c in range(NCORES)], axis=1)
    cs_out = np.concatenate([res.results[c]["cs_out"] for c in range(NCORES)], axis=1)
    return (h_last, hs_out, cs_out)
